# revision 1
# baseline (speedup 1.0000x reference)
"""Trainium2 Bass kernel for windowed attention with relative-position bias.

Problem (hardcoded shapes):
  x        [16, 1024, 256] f32
  w_qkv    [256, 768]      f32
  w_proj   [256, 256]      f32
  b_proj   [256]           f32
  bias_table [3969, 8]     f32
  out      [16, 1024, 256] f32

Sharding: data-parallel over batch B=16 across 8 cores (2 batches/core).
Weights / bias table replicated.

The relative-position bias matrix bias[m, n] = tbl[yn-ym+31, xn-xm+31]
(m = key token (ym, xm), n = query token (yn, xn) on the 32x32 grid) is
block-Toeplitz, so instead of uploading the materialized exp-bias
[8, 1024, 1024] (16 MB/core/exec), the kernel uploads the exp'd 63x63
table per head (62 KB) and gathers bias tiles on-device with strided
DMA reads (overlapping-window access patterns).

Key-token x-reversal: DMA access patterns require non-negative strides
on the partition-mapped dim, so keys are processed in (ym, 31-xm)
order.  Attention is permutation-invariant over keys (softmax + AV both
contract over m), so only the K/V construction switches to a reversed
copy of x^T; queries, output rows, and the final projection stay in
natural order.  With reversed keys the bias tile read is
  tile[p=(ym_l, xm'), (yn, xn)] = tbl[31-ym+yn, xm'+xn]
-> AP dims [(1, 32), (63, 32), (1, 32)]: all positive.

Per-core device algorithm (per batch b in 0..1, heads H=8, D=32, N=1024):
  qkT   = w_qk.T @ x^T            [512, 1024]  (q rows scaled by D^-0.5,
                                   k rows from reversed-key x^T)
  V     = xk @ w_v                [1024', 256] reversed-key row order
  per head pair hp:
    S^T[m',n]  = sum_d kT[d,m'] qT[d,n]         (K=32 row-packed matmuls)
    P^T        = exp(S^T) * expB_tile           (ACT exp + DVE mult)
    O^T_aug    = V_aug.T @ P^T  accumulated over m-chunks
                 (denominator rows via ones column)
  normalize via PE broadcast + reciprocal, y = O'^T.T @ w_proj + b_proj.
"""

import numpy as np
import ml_dtypes
import bass_rust

import concourse.bass as bass
import concourse.mybir as mybir
import concourse.tile as tile
from concourse import bacc
from concourse.bass_utils import run_bass_kernel_spmd

BF16 = mybir.dt.bfloat16
F32 = mybir.dt.float32

B, N, C = 16, 1024, 256
H, D = 8, 32
SCALE = D ** -0.5
H_GRID = W_GRID = 32
TS = 63  # table side
TABLE_SIZE = TS * TS  # 3969
N_CORES = 8
B_PER_CORE = B // N_CORES  # 2

_nbf = ml_dtypes.bfloat16


def build_nc():
    nc = bacc.Bacc("TRN2", target_bir_lowering=False, debug=False,
                   num_devices=N_CORES)

    xt = nc.dram_tensor("xt", [B_PER_CORE, C, N], BF16, kind="ExternalInput").ap()
    wqk = nc.dram_tensor("wqk", [C, 512], BF16, kind="ExternalInput").ap()
    wv = nc.dram_tensor("wv", [C, 256], BF16, kind="ExternalInput").ap()
    wproj = nc.dram_tensor("wproj", [C, 256], BF16, kind="ExternalInput").ap()
    # b_proj pre-tiled twice: [1, 512] = [b_proj, b_proj]
    bproj = nc.dram_tensor("bproj", [1, 512], BF16, kind="ExternalInput").ap()
    # exp'd bias table per head, y-REVERSED rows: tblx[h, rr, c] =
    # exp(tbl[h, 62-rr, c]), flat [H * 63 * 63]
    tblx = nc.dram_tensor("tblx", [H * TABLE_SIZE], BF16, kind="ExternalInput").ap()
    # x-expanded scratch: tblxp[h, rr, xm', xn] = tblx[h, rr, xm'+xn]
    # (so tblxp[h, 31+4m+ym_l-yn, xm', xn] = exp bias for key (ym, 31-xm'),
    #  query (yn, xn) of m-tile m) -> a single affine DMA per bias tile.
    tblxp = nc.dram_tensor("tblxp", [H * TS * 32 * 32], BF16, kind="Internal").ap()
    # ebc rows 0,32,64,96 are all-ones, everything else zero (broadcast lhsT)
    ebc = nc.dram_tensor("ebc", [128, 32], BF16, kind="ExternalInput").ap()
    y = nc.dram_tensor("y", [B_PER_CORE, N, C], BF16, kind="ExternalOutput").ap()

    from contextlib import ExitStack
    with tile.TileContext(nc) as tc, ExitStack() as ctx:
        consts = ctx.enter_context(tc.tile_pool(name="consts", bufs=1))
        persist = ctx.enter_context(tc.tile_pool(name="persist", bufs=1))
        xt_pool = ctx.enter_context(tc.tile_pool(name="xt", bufs=1))
        bias_pool = ctx.enter_context(tc.tile_pool(name="bias", bufs=2))
        pt_pool = ctx.enter_context(tc.tile_pool(name="pt", bufs=4))
        rec_pool = ctx.enter_context(tc.tile_pool(name="rec", bufs=2))
        ysb_pool = ctx.enter_context(tc.tile_pool(name="ysb", bufs=4))
        s_psum = ctx.enter_context(tc.tile_pool(name="spsum", bufs=3, space="PSUM"))
        av_psum = ctx.enter_context(tc.tile_pool(name="avpsum", bufs=1, space="PSUM"))

        # ---- inputs needed for the first block only; everything not used
        # until later (xT batch 1, wproj, bproj) is deferred behind hp0's
        # bias tiles so those win the DMA queue race against the m-loop ----
        xt_tiles = []
        xt_sb0 = xt_pool.tile([128, 2, N], BF16, name="xt0")
        nc.sync.dma_start(xt_sb0[:],
                          xt[0].rearrange("(kc p) n -> p kc n", p=128))
        xt_tiles.append(xt_sb0)
        wqk_sb = consts.tile([128, 2, 512], BF16)
        nc.sync.dma_start(wqk_sb[:], wqk.rearrange("(kc p) m -> p kc m", p=128))
        wv_sb = consts.tile([128, 2, 256], BF16)
        nc.sync.dma_start(wv_sb[:], wv.rearrange("(kc p) m -> p kc m", p=128))
        ebc_sb = consts.tile([128, 32], BF16)
        nc.sync.dma_start(ebc_sb[:], ebc)
        # warm the ACT exp table early
        actwarm = consts.tile([128, 8], F32)
        nc.scalar.activation(actwarm[:], ebc_sb[:, 0:8],
                             mybir.ActivationFunctionType.Exp)
        ones1 = consts.tile([1, 128], BF16)
        nc.gpsimd.memset(ones1[:], 1.0)
        wproj_sb = consts.tile([128, 2, 256], BF16)
        bproj_sb = consts.tile([1, 512], BF16)
        bproj2_sb = consts.tile([128, 512], F32)

        def load_deferred_inputs():
            # wproj/bproj aren't read until the hp3 epilogue
            nc.sync.dma_start(wproj_sb[:],
                              wproj.rearrange("(kc p) c -> p kc c", p=128))
            nc.sync.dma_start(bproj_sb[:], bproj)
            # broadcast b_proj to all 128 partitions via PE outer product
            bp_ps = s_psum.tile([128, 512], F32, tag="sps", name="bp_ps")
            nc.tensor.matmul(bp_ps[:], lhsT=ones1[:], rhs=bproj_sb[:],
                             start=True, stop=True)
            nc.vector.tensor_copy(bproj2_sb[:], bp_ps[:])

        # x-expand the bias table into DRAM scratch ((h, rr) merged: per-head
        # stride 3969 = 63*63).  SP DMAs share one FIFO queue, so each
        # expansion completes before that head-pair's bias-tile reads, and
        # per-hp expansion keeps hp0's first tiles near the queue head.
        def expand_heads(hp):
            nc.sync.dma_start(
                tblxp[2 * hp * TS * 1024:(2 * hp + 2) * TS * 1024],
                bass_rust.AP(tblx.tensor, 2 * hp * TABLE_SIZE,
                             [(TS, 2 * TS), (1, 32), (1, 32)]))

        # batch 1 input right behind batch 0's
        xt_sb1 = xt_pool.tile([128, 2, N], BF16, name="xt1")
        nc.sync.dma_start(xt_sb1[:],
                          xt[1].rearrange("(kc p) n -> p kc n", p=128))
        xt_tiles.append(xt_sb1)
        # reversed-key copies of x^T: token (ym, xm) -> (ym, 31-xm)
        xtk_tiles = []
        for b in range(B_PER_CORE):
            xtk_sb = xt_pool.tile([128, 2, N], BF16, name=f"xtk{b}")
            src = xt_tiles[b].rearrange("p kc (ym xm) -> p kc ym xm", xm=32)[
                :, :, :, ::-1]
            nc.vector.tensor_copy(
                xtk_sb.rearrange("p kc (ym xm) -> p kc ym xm", xm=32), src)
            xtk_tiles.append(xtk_sb)

        # persistent per-batch tensors
        qk_sb = persist.tile([128, B_PER_CORE, 4, N], BF16)  # [p, b, mtile, n]
        v_sb = persist.tile([128, B_PER_CORE, 8, H * 64], BF16)  # [p, b, mchunk, h*64+c]
        ot_sb = persist.tile([128, B_PER_CORE, 4, N], BF16)  # [p, b, hp, n]
        ot_remap = persist.tile([128, B_PER_CORE, 2, N], BF16)  # [inner%128, b, kc, n]
        # denominator rows DMA-packed at partitions 0/32/64/96 per (b, kc)
        dpack = persist.tile([128, B_PER_CORE, 2, N], BF16)
        # V_aug head slots are 64 wide: cols 0-31 = V_h, col 32 = ones,
        # cols 33-63 = zero (AV writes full 64-row halves).  These memsets run
        # on the (otherwise idle) GPSIMD engine, off the DVE critical path.
        nc.gpsimd.memset(v_sb[:], 0.0)
        ones_view = v_sb.rearrange("p b m (h c) -> p b m h c", c=64)[:, :, :, :, 32:33]
        nc.gpsimd.memset(ones_view, 1.0)
        nc.gpsimd.memset(dpack[:], 0.0)

        # ---- Phase A: qkT and V (emitted per batch; b1 is emitted after
        # the first attention block so the ACT pipeline starts early) ----
        def phase_a_qk(b, mts=(0, 2, 1, 3)):
            # qkT: lhsT = wqk [256, 512] chunks, rhs = xT -> out [512, 1024]
            # q tiles (mt 0,1) read natural xT; k tiles (mt 2,3) read the
            # reversed-key xT so kT columns follow the m' ordering.
            for mt in mts:
                rhs_sb = xt_tiles[b] if mt < 2 else xtk_tiles[b]
                ps = s_psum.tile([128, 1024], F32, tag="sps", name="ps")
                for nchk in range(2):
                    for kc in range(2):
                        nc.tensor.matmul(
                            ps[:, nchk * 512:(nchk + 1) * 512],
                            lhsT=wqk_sb[:, kc, mt * 128:(mt + 1) * 128],
                            rhs=rhs_sb[:, kc, nchk * 512:(nchk + 1) * 512],
                            start=(kc == 0), stop=(kc == 1),
                        )
                nc.vector.tensor_copy(qk_sb[:, b, mt, :], ps[:])

        def phase_a_v(b, groups=(0, 1)):
            # V: lhsT = xkT chunks [128, ntile], rhs = wv -> out [ntile, 256]
            # V rows in reversed-key order to match S^T row order.
            xtk_sb = xtk_tiles[b]
            for g in groups:  # groups of 4 m-chunks
                vp = s_psum.tile([128, 1024], F32, tag="sps", name="vp")
                for nt in range(4):
                    for kc in range(2):
                        nc.tensor.matmul(
                            vp[:, nt * 256:(nt + 1) * 256],
                            lhsT=xtk_sb[:, kc, (4 * g + nt) * 128:(4 * g + nt + 1) * 128],
                            rhs=wv_sb[:, kc, :],
                            start=(kc == 0), stop=(kc == 1),
                        )
                vsrc = vp.rearrange("p (nt h c) -> p nt h c", nt=4, h=8)
                vdst = v_sb.rearrange("p b m (h c) -> p b m h c", c=64)[
                    :, b, 4 * g:4 * g + 4, :, 0:32]
                nc.vector.tensor_copy(vdst, vsrc)

        def load_bias_tile(h, m):
            # tile[p=(ym_l, xm'), (yn, xn)] = exp bias of key (ym, 31-xm'),
            # query (yn, xn) = tblxp[h, 31+4m+ym_l-yn, xm', xn]:
            # offset affine in p = 32*ym_l + xm' -> ONE DMA per tile.
            btile = bias_pool.tile([128, N], BF16,
                                   tag=f"bias_{h % 2}_{m}",
                                   name=f"bias_{h % 2}_{m}")
            off = h * TS * 1024 + 1024 * (31 + 4 * m)
            src = bass_rust.AP(tblxp.tensor, off,
                               [(32, 128), (-1024, 32), (1, 32)])
            nc.sync.dma_start(btile[:], src)
            return btile

        def attention_block(hp, b, bias_tiles, after_m=None, tail=False):
            h0, h1 = 2 * hp, 2 * hp + 1
            t = h0 // 4
            av = av_psum.tile([128, 1024], F32, name=f"av{b}", tag="avpsum")
            for m in range(8):
                pair = ((0, h0), (1, h1))
                sp = {}
                # QK^T for both heads back-to-back: the two K=32 matmuls
                # per nchk sit in distinct PE row groups.
                for nchk in range(2):
                    sl = slice(nchk * 512, (nchk + 1) * 512)
                    for hi, h in pair:
                        bp = 32 * (h % 4)
                        if nchk == 0:
                            sp[hi] = s_psum.tile([128, 1024], F32,
                                                 tag="sps", name=f"sp{hi}")
                        nc.tensor.matmul(
                            sp[hi][:, sl],
                            lhsT=qk_sb[bp:bp + 32, b, 2 + t, m * 128:(m + 1) * 128],
                            rhs=qk_sb[bp:bp + 32, b, t, sl],
                            start=True, stop=True,
                            tile_position=(bp, 0),
                        )
                pt = {}
                for hi, h in pair:
                    pt[hi] = pt_pool.tile([128, 1024], BF16,
                                          tag=f"pt{hi}", name=f"pt{hi}")
                    praw = pt_pool.tile([128, 1024], BF16,
                                        tag=f"praw{hi}", name=f"praw{hi}")
                    nc.scalar.activation(
                        praw[:], sp[hi][:],
                        mybir.ActivationFunctionType.Exp)
                    nc.vector.tensor_mul(
                        out=pt[hi][:], in0=praw[:],
                        in1=bias_tiles[(hi, m)][:])
                if after_m and ("pre_av", m) in after_m:
                    after_m[("pre_av", m)]()
                # AV accumulate: lhsT = V_aug head slot [128, 64];
                # hi0/hi1 target disjoint col groups.
                for nchk in range(2):
                    sl = slice(nchk * 512, (nchk + 1) * 512)
                    for hi, h in pair:
                        po = 64 * hi
                        nc.tensor.matmul(
                            av[po:po + 64, sl],
                            lhsT=v_sb[:, b, m, h * 64:(h + 1) * 64],
                            rhs=pt[hi][:, sl],
                            start=(m == 0), stop=(m == 7),
                            tile_position=(0, po),
                            skip_group_check=True,
                        )
                if after_m and m in after_m:
                    after_m[m]()
            # evacuate unnormalized O^T (denominator rows at partitions
            # 32 / 96 ride along); then remap O rows and denominators.
            nc.vector.tensor_copy(ot_sb[:, b, hp, :], av[:])
            for hi in range(2):
                h = 2 * hp + hi
                # in the final block the SP queue round-robins with the ACT
                # queue (idle during drain) so the four small copies don't
                # serialize behind one FIFO on the drain critical path.
                eng = nc.scalar if (tail and hi == 1) else nc.sync
                eng.dma_start(
                    ot_remap[(32 * h) % 128:(32 * h) % 128 + 32,
                             b, h // 4, :],
                    ot_sb[64 * hi:64 * hi + 32, b, hp, :])
                eng.dma_start(
                    dpack[(32 * h) % 128:(32 * h) % 128 + 1,
                          b, h // 4, :],
                    ot_sb[32 + 64 * hi:33 + 64 * hi, b, hp, :])

        def normalize(b, kc, split=False):
            # broadcast denominators to their 32-row head blocks, then
            # reciprocal and in-place normalize of ot_remap[:, b, kc, :].
            # split=True (drain only): per n-half, so the projection's first
            # n-tiles can start while the second half is still normalizing.
            halves = ((0, 512), (512, 1024)) if split else ((0, 1024),)
            for lo, hi in halves:
                rp = s_psum.tile([128, 1024], F32, tag="sps", name="rp")
                for k in range(4):
                    for nchk in range(2):
                        sl = slice(lo + nchk * (hi - lo) // 2,
                                   lo + (nchk + 1) * (hi - lo) // 2)
                        nc.tensor.matmul(
                            rp[32 * k:32 * k + 32, sl],
                            lhsT=ebc_sb[32 * k:32 * k + 32, :],
                            rhs=dpack[32 * k:32 * k + 32, b, kc, sl],
                            start=True, stop=True,
                            tile_position=(32 * k, 32 * k),
                        )
                rsb = rec_pool.tile([128, 1024], F32, tag="rsb", name="rsb")
                nc.vector.reciprocal(rsb[:, lo:hi], rp[:, lo:hi])
                nc.vector.tensor_mul(out=ot_remap[:, b, kc, lo:hi],
                                     in0=ot_remap[:, b, kc, lo:hi],
                                     in1=rsb[:, lo:hi])

        def phase_c(b):
            # output projection, n-tiles in pairs
            y_re = y[b].rearrange("(g p) c -> p g c", p=128)
            for ntp in range(4):
                ysb = ysb_pool.tile([128, 2, 256], BF16, name="ysb")
                # yp rotates through the (drain-idle) 3-deep sps ring, so
                # iteration n+1's matmuls don't wait on iteration n's add.
                yp = s_psum.tile([128, 512], F32, tag="sps", name="yp")
                for sub in range(2):
                    nt = 2 * ntp + sub
                    for kc in range(2):
                        nc.tensor.matmul(
                            yp[:, sub * 256:(sub + 1) * 256],
                            lhsT=ot_remap[:, b, kc, nt * 128:(nt + 1) * 128],
                            rhs=wproj_sb[:, kc, :],
                            start=(kc == 0), stop=(kc == 1),
                            skip_group_check=True,
                        )
                nc.vector.tensor_add(
                    out=ysb.rearrange("p g c -> p (g c)"), in0=yp[:],
                    in1=bproj2_sb[:])
                nc.sync.dma_start(y_re[:, 2 * ntp:2 * ntp + 2, :], ysb[:])

        import os
        REPEAT = int(os.environ.get("K_REPEAT", "1"))  # body repeats (timing)
        for _rep in range(REPEAT):
            phase_a_qk(0)
            for hp in range(4):
                if _rep == 0:
                    expand_heads(hp)
                # load order matches consumption order (m-major): the m-loop
                # needs both heads' tile m before either head's tile m+1.
                bias_tiles = {}
                for m in range(8):
                    for hi, h in ((0, 2 * hp), (1, 2 * hp + 1)):
                        bias_tiles[(hi, m)] = load_bias_tile(h, m)
                if _rep == 0 and hp == 1:
                    load_deferred_inputs()
                for b in range(B_PER_CORE):
                    # spread the rest of phase A through the first (ACT-
                    # bound) attention blocks, one psum-ring allocation per
                    # m-slot so the S^T ring never stalls on phase-A tiles.
                    if hp == 0 and b == 0:
                        attention_block(hp, b, bias_tiles, after_m={
                            ("pre_av", 0): lambda: phase_a_v(0),
                            2: lambda: phase_a_qk(1),
                            4: lambda: phase_a_v(1),
                        })
                    else:
                        attention_block(hp, b, bias_tiles,
                                        tail=(hp == 3 and b == 1))
                    # each batch's normalize runs in the shadow of the other
                    # batch's ACT-bound attention block.  b0's projection is
                    # deferred past b1's block so its DVE work doesn't delay
                    # b1's bias-mult -> AV stream (which paces the drain).
                    if hp == 1:
                        normalize(b, 0)
                # the whole hp3 epilogue runs after both blocks: b0's (ready)
                # normalize/projection fills the wait on b1's remap DMAs.
                if hp == 3:
                    normalize(0, 1)
                    phase_c(0)
                    normalize(1, 1)
                    phase_c(1)

    nc.compile()
    return nc


_NC_CACHE = None


def _get_nc():
    global _NC_CACHE
    if _NC_CACHE is None:
        _NC_CACHE = build_nc()
    return _NC_CACHE


def _host_prep(x, w_qkv, w_proj, b_proj, bias_table):
    # exp'd bias table per head, rows y-reversed:
    # tblx[h, rr, c] = exp(bias_table[63*(62-rr) + c, h])
    tblx = np.ascontiguousarray(
        np.exp(bias_table.astype(np.float32)).T.reshape(H, TS, TS)[:, ::-1, :]
    ).astype(_nbf)

    xt = np.ascontiguousarray(np.transpose(x, (0, 2, 1))).astype(_nbf)  # [B, C, N]
    w_qk = np.concatenate(
        [w_qkv[:, :256] * SCALE, w_qkv[:, 256:512]], axis=1).astype(_nbf)
    w_v = w_qkv[:, 512:].astype(_nbf)
    wproj_arr = w_proj.astype(_nbf)
    bproj_arr = np.concatenate([b_proj, b_proj]).reshape(1, 512).astype(_nbf)
    ebc = np.zeros((128, 32), dtype=_nbf)
    for p in (0, 32, 64, 96):
        ebc[p, :] = 1.0
    return xt, w_qk, w_v, wproj_arr, bproj_arr, tblx, ebc


def kernel(x, w_qkv, w_proj, b_proj, bias_table):
    x = np.asarray(x, dtype=np.float32)
    w_qkv = np.asarray(w_qkv, dtype=np.float32)
    w_proj = np.asarray(w_proj, dtype=np.float32)
    b_proj = np.asarray(b_proj, dtype=np.float32)
    bias_table = np.asarray(bias_table, dtype=np.float32)

    xt, w_qk, w_v, wproj_arr, bproj_arr, tblx, ebc = _host_prep(
        x, w_qkv, w_proj, b_proj, bias_table)

    nc = _get_nc()
    in_maps = []
    for c in range(N_CORES):
        in_maps.append({
            "xt": xt[B_PER_CORE * c:B_PER_CORE * (c + 1)],
            "wqk": w_qk, "wv": w_v, "wproj": wproj_arr, "bproj": bproj_arr,
            "tblx": tblx.ravel(), "ebc": ebc,
        })
    res = run_bass_kernel_spmd(nc, in_maps, core_ids=list(range(N_CORES)))
    out = np.concatenate([res.results[c]["y"] for c in range(N_CORES)], axis=0)
    return out.astype(np.float32)


if __name__ == "__main__":
    rng = np.random.default_rng(0)
    inputs = {
        "x": rng.standard_normal((B, N, C), dtype=np.float32),
        "w_qkv": (rng.standard_normal((C, 3 * 256), dtype=np.float32) * C ** -0.5),
        "w_proj": (rng.standard_normal((256, C), dtype=np.float32) * 256 ** -0.5),
        "b_proj": np.zeros((C,), dtype=np.float32),
        "bias_table": (rng.standard_normal((TABLE_SIZE, H), dtype=np.float32) * 0.02),
    }
    out = kernel(**inputs)
    print("kernel output", out.shape, out.dtype)

